# revision 14
# baseline (speedup 1.0000x reference)
"""GCN (2x GCNConv + linear head) on 8 NeuronCores via Bass/Tile.

v4 strategy (graph/data parallel, fp8 DoubleRow both layers):
  - Nodes padded to 10240 = 80 blocks of 128; core c owns dst range
    [c*1280, (c+1)*1280).
  - S = D^-1/2 (A+I) D^-1/2: A holds exact edge counts as dense fp8
    blocks; per-node scalings host-precomputed (g0 = dinv*x in fp8,
    dinvb broadcast tile).
  - Both aggregations run DoubleRow (paired 128-src-blocks, both
    operands fp8) at 2x PE rate. Features quantized to fp8 cost
    rel_err ~1.8e-2 (simulated) vs the 2e-2 gate; L2FP8=False falls
    back to bf16 layer 2 (~8e-3).
  - dinv_dst folded into the PSUM->SBUF cast (commutes with the W
    contraction), relu reads PSUM directly: short agg->cc_in chain.
  - g1 transposed to node-major BEFORE the collective; 3 pipelined
    AllGathers (one per dst chunk) with ~20us trigger->mesh latency
    hidden under layer-2 compute of earlier chunks.
  - A streamed in consumption order, split across both HWDGE rings.
  - Head accumulates all 10 blocks into one PSUM bank; output stored
    partition-major ([128, 10*C]) so the final DMA is contiguous and
    the host unpacks.
"""
import numpy as np
import ml_dtypes

import concourse.bass as bass
import concourse.mybir as mybir
import concourse.tile as tile
import concourse.bacc as bacc
from concourse.bass_utils import run_bass_kernel_spmd

FP8 = np.dtype(ml_dtypes.float8_e4m3)
BF16 = np.dtype(ml_dtypes.bfloat16)

N, E, D, C = 10000, 640000, 128, 40
NCORES = 8
NSB = 80                      # src blocks of 128
NPAD = NSB * 128              # 10240
DST = NPAD // NCORES          # 1280 dst nodes per core
CH = [(0, 512), (512, 512), (1024, 256)]
NHK = DST // 128              # head chunks of 128
L2FP8 = False                 # fp8 DoubleRow layer-2 aggregation: ~3.5us
                              # faster but rel_err 1.8e-2 vs 8e-3 (gate 2e-2)

_cache = {}


def _build():
    nc = bacc.Bacc("TRN2", target_bir_lowering=False, debug=False,
                   num_devices=NCORES)
    f32 = mybir.dt.float32
    bf16 = mybir.dt.bfloat16
    fp8 = mybir.dt.float8e4
    RELU = mybir.ActivationFunctionType.Relu
    DR = mybir.MatmulPerfMode.DoubleRow
    g1dt, g1np = (fp8, FP8) if L2FP8 else (bf16, BF16)

    g0_d = nc.dram_tensor("g0", [128, NPAD], fp8, kind="ExternalInput")
    W1b = nc.dram_tensor("W1b", [D, D], bf16, kind="ExternalInput")
    W2b = nc.dram_tensor("W2b", [D, D], bf16, kind="ExternalInput")
    Wh = nc.dram_tensor("Wh", [D, C], f32, kind="ExternalInput")
    b1 = nc.dram_tensor("b1", [D, 1], f32, kind="ExternalInput")
    b2 = nc.dram_tensor("b2", [D, 1], f32, kind="ExternalInput")
    bhb_d = nc.dram_tensor("bhb", [128, C], f32, kind="ExternalInput")
    eye_d = nc.dram_tensor("eye", [128, 128], bf16, kind="ExternalInput")
    dinvb_d = nc.dram_tensor("dinvb", [128, DST], f32, kind="ExternalInput")
    A_d = [nc.dram_tensor(f"A{ci}", [128, NSB * ln], fp8, kind="ExternalInput")
           for ci, (off, ln) in enumerate(CH)]
    out = nc.dram_tensor("out", [128, NHK * C], f32, kind="ExternalOutput")

    with tile.TileContext(nc) as tc:
        with (
            tc.tile_pool(name="big", bufs=1) as big,
            tc.tile_pool(name="sb", bufs=1) as sb,
            tc.tile_pool(name="tmpp", bufs=3) as tmpp,
            tc.tile_pool(name="psagg", bufs=3, space="PSUM") as psagg,
            tc.tile_pool(name="psz", bufs=1, space="PSUM") as psz,
            tc.tile_pool(name="pstr", bufs=2, space="PSUM") as pstr,
            tc.tile_pool(name="pshd", bufs=1, space="PSUM") as pshd,
            tc.tile_pool(name="dram", bufs=1, space="DRAM") as dram,
        ):
            # (no warm-up collective: the first mesh starts at a fixed
            # ~77us firmware-bootstrap floor regardless; an extra
            # collective only adds serial mesh time in front of the
            # real gathers)

            # ---- all input streams issued up front, consumption order,
            # split across both HWDGE rings (compute instructions later in
            # program order would head-of-line-block the ring queues) ----
            A_t = [big.tile([128, NSB * ln], fp8, name=f"At{ci}")
                   for ci, (off, ln) in enumerate(CH)]
            g0 = big.tile([128, NPAD], fp8)
            for p in range(4):  # even pieces -> sync, odd -> scalar
                s0, s1 = p * (NPAD // 4), (p + 1) * (NPAD // 4)
                (nc.sync if p % 2 == 0 else nc.scalar).dma_start(
                    g0[:, s0:s1], g0_d[:, s0:s1])
            dinvb = sb.tile([128, DST], f32)
            nc.scalar.dma_start(dinvb[:], dinvb_d[:, :])
            W1_t = sb.tile([D, D], bf16)
            nc.scalar.dma_start(W1_t[:], W1b[:, :])
            b1_t = sb.tile([D, 1], f32)
            nc.scalar.dma_start(b1_t[:], b1[:, :])
            eye_t = sb.tile([128, 128], bf16)
            nc.scalar.dma_start(eye_t[:], eye_d[:, :])

            def stream_A(ci, npc):
                ln = CH[ci][1]
                for q in range(npc):
                    s0 = q * (NSB // npc) * ln
                    s1 = (q + 1) * (NSB // npc) * ln
                    (nc.sync if q % 2 == 0 else nc.scalar).dma_start(
                        A_t[ci][:, s0:s1], A_d[ci][:, s0:s1])

            stream_A(0, 8)
            stream_A(1, 4)
            stream_A(2, 4)
            W2_t = sb.tile([D, D], bf16)
            nc.scalar.dma_start(W2_t[:], W2b[:, :])
            b2_t = sb.tile([D, 1], f32)
            nc.scalar.dma_start(b2_t[:], b2[:, :])
            Wh_t = sb.tile([D, C], f32)
            nc.scalar.dma_start(Wh_t[:], Wh[:, :])
            bhb = sb.tile([128, C], f32)
            nc.scalar.dma_start(bhb[:], bhb_d[:, :])

            # ---- layer 1 per chunk -> transpose -> allgather ----
            # collectives carry bf16 (fp8 collectives measured broken/slow)
            cc_src = sb.tile([128, DST], bf16)     # node-major g1
            cc_io = []

            def l1_chunk(ci):
                off, ln = CH[ci]
                agg = psagg.tile([128, 512], f32, tag="agg", name=f"agg1_{ci}")
                for pb in range(NSB // 2):
                    nc.tensor.matmul(
                        agg[:, :ln],
                        lhsT=g0[:, pb * 256:(pb + 1) * 256].rearrange(
                            "p (two f) -> p two f", two=2),
                        rhs=A_t[ci][:, pb * 2 * ln:(pb * 2 + 2) * ln].rearrange(
                            "p (two n) -> p two n", two=2),
                        start=(pb == 0), stop=(pb == NSB // 2 - 1),
                        perf_mode=DR)
                # dinv_dst folded here (commutes with the W1 contraction)
                acc = tmpp.tile([128, 512], bf16, tag="acc")
                nc.vector.tensor_mul(acc[:, :ln], agg[:, :ln],
                                     dinvb[:, off:off + ln])
                zps = psz.tile([128, 512], f32, tag="z")
                nc.tensor.matmul(zps[:, :ln], lhsT=W1_t[:], rhs=acc[:, :ln],
                                 start=True, stop=True)
                t2 = tmpp.tile([128, 512], f32, tag="t2")
                nc.scalar.activation(t2[:, :ln], zps[:, :ln], RELU,
                                     bias=b1_t[:, 0:1], scale=1.0)
                g1c = tmpp.tile([128, 512], bf16, tag="g1c")
                nc.vector.tensor_mul(g1c[:, :ln], t2[:, :ln],
                                     dinvb[:, off:off + ln])
                nt = ln // 128
                # transpose in bf16 (fp8 PE-transpose needs stride-2 out);
                # the PSUM->SBUF copy below converts to g1dt
                trp = pstr.tile([128, 4, 128], bf16, tag="tr")
                for t in range(nt):
                    nc.tensor.transpose(trp[:, t, :],
                                        g1c[:, t * 128:(t + 1) * 128], eye_t[:])
                nc.vector.tensor_copy(
                    cc_src[:, off:off + ln].rearrange(
                        "p (t f) -> p t f", f=128),
                    trp[:, :nt, :])
                # fire this chunk's allgather (cc_in write on the sync ring)
                cc_in = dram.tile([128, ln], bf16, name=f"cc_in{ci}")
                cc_out = dram.tile([NCORES, 128, ln], bf16,
                                   addr_space="Shared", name=f"cc_out{ci}")
                nc.sync.dma_start(cc_in[:], cc_src[:, off:off + ln])
                nc.gpsimd.collective_compute(
                    "AllGather", mybir.AluOpType.bypass,
                    replica_groups=[list(range(NCORES))],
                    ins=[cc_in[:]], outs=[cc_out[:]])
                cc_io.append((cc_in, cc_out))

            l1_chunk(0)
            l1_chunk(1)
            l1_chunk(2)

            # ---- layer 2: gathered node-major slabs feed matmuls ----
            agg2 = [psagg.tile([128, 512], f32, tag="agg", name=f"agg2_{c2}")
                    for c2 in range(len(CH))]
            slabs = []
            for ci, (off, ln) in enumerate(CH):
                slab = sb.tile([128, NCORES * ln], bf16, name=f"slab{ci}")
                H = NCORES // 2
                for hh in range(2):
                    nc.sync.dma_start(
                        slab[:, hh * H * ln:(hh + 1) * H * ln].rearrange(
                            "p (r d) -> p r d", d=ln),
                        cc_io[ci][1][hh * H:(hh + 1) * H, :, :].rearrange(
                            "r p d -> p r d"))
                if L2FP8:
                    # DoubleRow needs fp8 operands: convert on DVE, in two
                    # halves so layer-2 matmuls unblock per half
                    slab8 = sb.tile([128, NCORES * ln], fp8,
                                    name=f"slab8_{ci}")
                    for hh in range(2):
                        sl = slice(hh * H * ln, (hh + 1) * H * ln)
                        nc.vector.tensor_copy(slab8[:, sl], slab[:, sl])
                    slabs.append(slab8)
                else:
                    slabs.append(slab)

            # per-chunk block tables: global src block for (ci, r, t)
            k = 0
            nblk = [ln // 128 for _, ln in CH]
            boff = [0, nblk[0], nblk[0] + nblk[1]]
            if L2FP8:
                # paired blocks within a core's slab segment (nblk even
                # for chunks 0/1; chunk 2 has 2 blocks = 1 pair)
                npairs = NSB // 2
                for ci, (off, ln) in enumerate(CH):
                    for r in range(NCORES):
                        for u in range(nblk[ci] // 2):
                            lhsT = slabs[ci][:, r * ln + u * 256:
                                             r * ln + (u + 1) * 256].rearrange(
                                "p (two f) -> p two f", two=2)
                            sb_g = r * NHK + boff[ci] + 2 * u
                            for c2, (off2, ln2) in enumerate(CH):
                                nc.tensor.matmul(
                                    agg2[c2][:, :ln2], lhsT=lhsT,
                                    rhs=A_t[c2][:, sb_g * ln2:
                                                (sb_g + 2) * ln2].rearrange(
                                        "p (two n) -> p two n", two=2),
                                    start=(k == 0), stop=(k == npairs - 1),
                                    perf_mode=DR)
                            k += 1
            else:
                for ci, (off, ln) in enumerate(CH):
                    for r in range(NCORES):
                        for t in range(nblk[ci]):
                            lhsT = slabs[ci][:, r * ln + t * 128:
                                             r * ln + (t + 1) * 128]
                            sb_g = r * NHK + boff[ci] + t
                            for c2, (off2, ln2) in enumerate(CH):
                                nc.tensor.matmul(
                                    agg2[c2][:, :ln2], lhsT=lhsT,
                                    rhs=A_t[c2][:, sb_g * ln2:(sb_g + 1) * ln2],
                                    start=(k == 0), stop=(k == NSB - 1))
                            k += 1

            h2 = sb.tile([128, DST], f32)
            for c2, (off2, ln2) in enumerate(CH):
                acc = tmpp.tile([128, 512], bf16, tag="acc")
                nc.vector.tensor_mul(acc[:, :ln2], agg2[c2][:, :ln2],
                                     dinvb[:, off2:off2 + ln2])
                zps = psz.tile([128, 512], f32, tag="z")
                nc.tensor.matmul(zps[:, :ln2], lhsT=W2_t[:], rhs=acc[:, :ln2],
                                 start=True, stop=True)
                nc.scalar.activation(h2[:, off2:off2 + ln2], zps[:, :ln2],
                                     RELU, bias=b2_t[:, 0:1], scale=1.0)

            # ---- head: all 10 blocks into one PSUM bank ----
            hd = pshd.tile([128, NHK * C], f32)
            for hk in range(NHK):
                nc.tensor.matmul(hd[:, hk * C:(hk + 1) * C],
                                 lhsT=h2[:, hk * 128:(hk + 1) * 128],
                                 rhs=Wh_t[:], start=True, stop=True)
            out_sb = sb.tile([128, NHK * C], f32)
            nc.vector.tensor_add(
                out_sb[:].rearrange("p (t c) -> p t c", c=C),
                hd[:].rearrange("p (t c) -> p t c", c=C),
                bhb[:].unsqueeze(1).broadcast_to([128, NHK, C]))
            nc.scalar.dma_start(out[:, :], out_sb[:])
    nc.compile()
    return nc


def _prep(x, edge_index, W1, b1, W2, b2, Wh, bh):
    x = np.asarray(x, np.float32)
    ei = np.asarray(edge_index, np.int64)
    src = np.concatenate([ei[0], np.arange(NPAD, dtype=np.int64)])
    dst = np.concatenate([ei[1], np.arange(NPAD, dtype=np.int64)])
    deg = np.bincount(dst, minlength=NPAD).astype(np.float32)
    dinv = 1.0 / np.sqrt(deg)

    xp = np.zeros((NPAD, D), np.float32)
    xp[:N] = x
    g0 = dinv[:, None] * xp
    g0_nm = g0.reshape(NSB, 128, D).transpose(1, 0, 2).reshape(128, NPAD)

    shared = {
        "g0": g0_nm.astype(FP8),
        "W1b": np.asarray(W1, np.float32).astype(BF16),
        "W2b": np.asarray(W2, np.float32).astype(BF16),
        "Wh": np.asarray(Wh, np.float32),
        "b1": np.asarray(b1, np.float32).reshape(D, 1),
        "b2": np.asarray(b2, np.float32).reshape(D, 1),
        "bhb": np.broadcast_to(np.asarray(bh, np.float32).reshape(1, C),
                               (128, C)).copy(),
        "eye": np.eye(128, dtype=np.float32).astype(BF16),
    }
    core = dst // DST
    sl, sbk = src % 128, src // 128
    in_maps = []
    for c in range(NCORES):
        m = core == c
        dloc = dst[m] - c * DST
        im = dict(shared, dinvb=np.broadcast_to(
            dinv[c * DST:(c + 1) * DST].reshape(1, DST), (128, DST)).copy())
        for ci, (off, ln) in enumerate(CH):
            m2 = (dloc >= off) & (dloc < off + ln)
            Ac = np.zeros((128, NSB * ln), np.float32)
            np.add.at(Ac, (sl[m][m2], sbk[m][m2] * ln + dloc[m2] - off), 1.0)
            im[f"A{ci}"] = Ac.astype(FP8)
        in_maps.append(im)
    return in_maps


def _run(inputs, trace=False):
    if "nc" not in _cache:
        _cache["nc"] = _build()
    in_maps = _prep(**inputs)
    res = run_bass_kernel_spmd(_cache["nc"], in_maps,
                               core_ids=list(range(NCORES)), trace=trace)
    # out is stored partition-major [128, NHK*C]; unpack to [DST, C]
    outs = []
    for c in range(NCORES):
        o = res.results[c]["out"].reshape(128, NHK, C)
        outs.append(o.transpose(1, 0, 2).reshape(DST, C))
    out = np.concatenate(outs, axis=0)[:N]
    return np.ascontiguousarray(out, dtype=np.float32), res


def kernel(**inputs):
    out, _ = _run(inputs, trace=False)
    return out


# revision 18
# speedup vs baseline: 1.1284x; 1.1284x over previous
"""GCN (2x GCNConv + linear head) on 8 NeuronCores via Bass/Tile.

v4 strategy (graph/data parallel, fp8 DoubleRow both layers):
  - Nodes padded to 10240 = 80 blocks of 128; core c owns dst range
    [c*1280, (c+1)*1280).
  - S = D^-1/2 (A+I) D^-1/2: A holds exact edge counts as dense fp8
    blocks; per-node scalings host-precomputed (g0 = dinv*x in fp8,
    dinvb broadcast tile).
  - Both aggregations run DoubleRow (paired 128-src-blocks, both
    operands fp8) at 2x PE rate. Features quantized to fp8 cost
    rel_err ~1.8e-2 (simulated) vs the 2e-2 gate; L2FP8=False falls
    back to bf16 layer 2 (~8e-3).
  - dinv_dst folded into the PSUM->SBUF cast (commutes with the W
    contraction), relu reads PSUM directly: short agg->cc_in chain.
  - g1 transposed to node-major BEFORE the collective; 3 pipelined
    AllGathers (one per dst chunk) with ~20us trigger->mesh latency
    hidden under layer-2 compute of earlier chunks.
  - A streamed in consumption order, split across both HWDGE rings.
  - Head accumulates all 10 blocks into one PSUM bank; output stored
    partition-major ([128, 10*C]) so the final DMA is contiguous and
    the host unpacks.
"""
import numpy as np
import ml_dtypes

import concourse.bass as bass
import concourse.mybir as mybir
import concourse.tile as tile
import concourse.bacc as bacc
from concourse.bass_utils import run_bass_kernel_spmd

FP8 = np.dtype(ml_dtypes.float8_e4m3)
BF16 = np.dtype(ml_dtypes.bfloat16)

N, E, D, C = 10000, 640000, 128, 40
NCORES = 8
NSB = 80                      # src blocks of 128
NPAD = NSB * 128              # 10240
DST = NPAD // NCORES          # 1280 dst nodes per core
CH = [(0, 512), (512, 512), (1024, 256)]
NHK = DST // 128              # head chunks of 128
L2FP8 = True                  # fp8 DoubleRow layer-2 aggregation: faster but
                              # rel_err 1.8e-2 vs 8e-3 (gate 2e-2, same seed)
AGS = [(0, 1024), (1024, 256)]  # allgather segments (cols of node-major g1)

_cache = {}


def _build():
    nc = bacc.Bacc("TRN2", target_bir_lowering=False, debug=False,
                   num_devices=NCORES)
    f32 = mybir.dt.float32
    bf16 = mybir.dt.bfloat16
    fp8 = mybir.dt.float8e4
    RELU = mybir.ActivationFunctionType.Relu
    DR = mybir.MatmulPerfMode.DoubleRow
    g1dt, g1np = (fp8, FP8) if L2FP8 else (bf16, BF16)

    g0_d = nc.dram_tensor("g0", [128, NPAD], fp8, kind="ExternalInput")
    W1b = nc.dram_tensor("W1b", [D, D], bf16, kind="ExternalInput")
    W2b = nc.dram_tensor("W2b", [D, D], bf16, kind="ExternalInput")
    Wh = nc.dram_tensor("Wh", [D, C], f32, kind="ExternalInput")
    b1 = nc.dram_tensor("b1", [D, 1], f32, kind="ExternalInput")
    b2 = nc.dram_tensor("b2", [D, 1], f32, kind="ExternalInput")
    bhb_d = nc.dram_tensor("bhb", [128, C], f32, kind="ExternalInput")
    eye_d = nc.dram_tensor("eye", [128, 128], bf16, kind="ExternalInput")
    dinvb_d = nc.dram_tensor("dinvb", [128, DST], f32, kind="ExternalInput")
    A_d = [nc.dram_tensor(f"A{ci}", [128, NSB * ln], fp8, kind="ExternalInput")
           for ci, (off, ln) in enumerate(CH)]
    out = nc.dram_tensor("out", [128, NHK * C], f32, kind="ExternalOutput")

    with tile.TileContext(nc) as tc:
        with (
            tc.tile_pool(name="big", bufs=1) as big,
            tc.tile_pool(name="sb", bufs=1) as sb,
            tc.tile_pool(name="tmpp", bufs=3) as tmpp,
            tc.tile_pool(name="psagg", bufs=3, space="PSUM") as psagg,
            tc.tile_pool(name="psz", bufs=1, space="PSUM") as psz,
            tc.tile_pool(name="pstr", bufs=2, space="PSUM") as pstr,
            tc.tile_pool(name="pshd", bufs=1, space="PSUM") as pshd,
            tc.tile_pool(name="dram", bufs=1, space="DRAM") as dram,
        ):
            # ---- warm-up collective: the first mesh starts at a fixed
            # ~77us firmware floor AND absorbs the ~14us cross-core
            # alignment; paying that inside a 128-byte mesh (~8us) beats
            # paying it inside the first real gather (measured +17us) ----
            warm_in = dram.tile([1, 128], bf16)
            warm_out = dram.tile([NCORES, 1, 128], bf16, addr_space="Shared")
            nc.gpsimd.collective_compute(
                "AllGather", mybir.AluOpType.bypass,
                replica_groups=[list(range(NCORES))],
                ins=[warm_in[:]], outs=[warm_out[:]])

            # ---- all input streams issued up front, consumption order,
            # split across both HWDGE rings (compute instructions later in
            # program order would head-of-line-block the ring queues) ----
            A_t = [big.tile([128, NSB * ln], fp8, name=f"At{ci}")
                   for ci, (off, ln) in enumerate(CH)]
            g0 = big.tile([128, NPAD], fp8)
            for p in range(4):  # even pieces -> sync, odd -> scalar
                s0, s1 = p * (NPAD // 4), (p + 1) * (NPAD // 4)
                (nc.sync if p % 2 == 0 else nc.scalar).dma_start(
                    g0[:, s0:s1], g0_d[:, s0:s1])
            dinvb = sb.tile([128, DST], f32)
            nc.scalar.dma_start(dinvb[:], dinvb_d[:, :])
            W1_t = sb.tile([D, D], bf16)
            nc.scalar.dma_start(W1_t[:], W1b[:, :])
            b1_t = sb.tile([D, 1], f32)
            nc.scalar.dma_start(b1_t[:], b1[:, :])
            eye_t = sb.tile([128, 128], bf16)
            nc.scalar.dma_start(eye_t[:], eye_d[:, :])

            def stream_A(ci, npc):
                ln = CH[ci][1]
                for q in range(npc):
                    s0 = q * (NSB // npc) * ln
                    s1 = (q + 1) * (NSB // npc) * ln
                    (nc.sync if q % 2 == 0 else nc.scalar).dma_start(
                        A_t[ci][:, s0:s1], A_d[ci][:, s0:s1])

            stream_A(0, 8)
            stream_A(1, 4)
            stream_A(2, 4)
            W2_t = sb.tile([D, D], bf16)
            nc.scalar.dma_start(W2_t[:], W2b[:, :])
            b2_t = sb.tile([D, 1], f32)
            nc.scalar.dma_start(b2_t[:], b2[:, :])
            Wh_t = sb.tile([D, C], f32)
            nc.scalar.dma_start(Wh_t[:], Wh[:, :])
            bhb = sb.tile([128, C], f32)
            nc.scalar.dma_start(bhb[:], bhb_d[:, :])

            # ---- layer 1 per chunk -> transpose -> allgather ----
            # collectives carry bf16 (fp8 collectives measured broken/slow)
            cc_src = sb.tile([128, DST], bf16)     # node-major g1
            cc_io = []

            def l1_chunk(ci):
                off, ln = CH[ci]
                agg = psagg.tile([128, 512], f32, tag="agg", name=f"agg1_{ci}")
                for pb in range(NSB // 2):
                    nc.tensor.matmul(
                        agg[:, :ln],
                        lhsT=g0[:, pb * 256:(pb + 1) * 256].rearrange(
                            "p (two f) -> p two f", two=2),
                        rhs=A_t[ci][:, pb * 2 * ln:(pb * 2 + 2) * ln].rearrange(
                            "p (two n) -> p two n", two=2),
                        start=(pb == 0), stop=(pb == NSB // 2 - 1),
                        perf_mode=DR)
                # dinv_dst folded here (commutes with the W1 contraction)
                acc = tmpp.tile([128, 512], bf16, tag="acc")
                nc.vector.tensor_mul(acc[:, :ln], agg[:, :ln],
                                     dinvb[:, off:off + ln])
                zps = psz.tile([128, 512], f32, tag="z")
                nc.tensor.matmul(zps[:, :ln], lhsT=W1_t[:], rhs=acc[:, :ln],
                                 start=True, stop=True)
                t2 = tmpp.tile([128, 512], f32, tag="t2")
                nc.scalar.activation(t2[:, :ln], zps[:, :ln], RELU,
                                     bias=b1_t[:, 0:1], scale=1.0)
                g1c = tmpp.tile([128, 512], bf16, tag="g1c")
                nc.vector.tensor_mul(g1c[:, :ln], t2[:, :ln],
                                     dinvb[:, off:off + ln])
                nt = ln // 128
                # transpose in bf16 (fp8 PE-transpose needs stride-2 out);
                # the PSUM->SBUF copy below converts to g1dt
                trp = pstr.tile([128, 4, 128], bf16, tag="tr")
                for t in range(nt):
                    nc.tensor.transpose(trp[:, t, :],
                                        g1c[:, t * 128:(t + 1) * 128], eye_t[:])
                nc.vector.tensor_copy(
                    cc_src[:, off:off + ln].rearrange(
                        "p (t f) -> p t f", f=128),
                    trp[:, :nt, :])

            def fire_ag(qi):
                o0, w = AGS[qi]
                cc_in = dram.tile([128, w], bf16, name=f"cc_in{qi}")
                cc_out = dram.tile([NCORES, 128, w], bf16,
                                   addr_space="Shared", name=f"cc_out{qi}")
                nc.sync.dma_start(cc_in[:], cc_src[:, o0:o0 + w])
                nc.gpsimd.collective_compute(
                    "AllGather", mybir.AluOpType.bypass,
                    replica_groups=[list(range(NCORES))],
                    ins=[cc_in[:]], outs=[cc_out[:]])
                cc_io.append((cc_in, cc_out))

            l1_chunk(0)
            l1_chunk(1)
            fire_ag(0)
            l1_chunk(2)
            fire_ag(1)

            # ---- layer 2: gathered node-major slabs feed matmuls ----
            agg2 = [psagg.tile([128, 512], f32, tag="agg", name=f"agg2_{c2}")
                    for c2 in range(len(CH))]
            slabs = []
            for qi, (o0, w) in enumerate(AGS):
                slab = sb.tile([128, NCORES * w], bf16, name=f"slab{qi}")
                H = NCORES // 2
                for hh in range(2):
                    nc.sync.dma_start(
                        slab[:, hh * H * w:(hh + 1) * H * w].rearrange(
                            "p (r d) -> p r d", d=w),
                        cc_io[qi][1][hh * H:(hh + 1) * H, :, :].rearrange(
                            "r p d -> p r d"))
                if L2FP8:
                    # DoubleRow needs fp8 operands: convert on DVE, in two
                    # halves so layer-2 matmuls unblock per half
                    slab8 = sb.tile([128, NCORES * w], fp8,
                                    name=f"slab8_{qi}")
                    for hh in range(2):
                        sl = slice(hh * H * w, (hh + 1) * H * w)
                        nc.vector.tensor_copy(slab8[:, sl], slab[:, sl])
                    slabs.append(slab8)
                else:
                    slabs.append(slab)

            # global src block for (qi, r, t): r*NHK + o0/128 + t
            k = 0
            nblk = [w // 128 for _, w in AGS]
            if L2FP8:
                npairs = NSB // 2
                for qi, (o0, w) in enumerate(AGS):
                    for r in range(NCORES):
                        for u in range(nblk[qi] // 2):
                            lhsT = slabs[qi][:, r * w + u * 256:
                                             r * w + (u + 1) * 256].rearrange(
                                "p (two f) -> p two f", two=2)
                            sb_g = r * NHK + o0 // 128 + 2 * u
                            for c2, (off2, ln2) in enumerate(CH):
                                nc.tensor.matmul(
                                    agg2[c2][:, :ln2], lhsT=lhsT,
                                    rhs=A_t[c2][:, sb_g * ln2:
                                                (sb_g + 2) * ln2].rearrange(
                                        "p (two n) -> p two n", two=2),
                                    start=(k == 0), stop=(k == npairs - 1),
                                    perf_mode=DR)
                            k += 1
            else:
                for qi, (o0, w) in enumerate(AGS):
                    for r in range(NCORES):
                        for t in range(nblk[qi]):
                            lhsT = slabs[qi][:, r * w + t * 128:
                                             r * w + (t + 1) * 128]
                            sb_g = r * NHK + o0 // 128 + t
                            for c2, (off2, ln2) in enumerate(CH):
                                nc.tensor.matmul(
                                    agg2[c2][:, :ln2], lhsT=lhsT,
                                    rhs=A_t[c2][:, sb_g * ln2:(sb_g + 1) * ln2],
                                    start=(k == 0), stop=(k == NSB - 1))
                            k += 1

            h2 = sb.tile([128, DST], f32)
            for c2, (off2, ln2) in enumerate(CH):
                acc = tmpp.tile([128, 512], bf16, tag="acc")
                nc.vector.tensor_mul(acc[:, :ln2], agg2[c2][:, :ln2],
                                     dinvb[:, off2:off2 + ln2])
                zps = psz.tile([128, 512], f32, tag="z")
                nc.tensor.matmul(zps[:, :ln2], lhsT=W2_t[:], rhs=acc[:, :ln2],
                                 start=True, stop=True)
                nc.scalar.activation(h2[:, off2:off2 + ln2], zps[:, :ln2],
                                     RELU, bias=b2_t[:, 0:1], scale=1.0)

            # ---- head: all 10 blocks into one PSUM bank; bias-add and
            # output DMA split in halves to pipeline the tail ----
            hd = pshd.tile([128, NHK * C], f32)
            out_sb = sb.tile([128, NHK * C], f32)
            HB = NHK // 2
            for hh in range(2):
                for hk in range(hh * HB, (hh + 1) * HB):
                    nc.tensor.matmul(hd[:, hk * C:(hk + 1) * C],
                                     lhsT=h2[:, hk * 128:(hk + 1) * 128],
                                     rhs=Wh_t[:], start=True, stop=True)
                sl = slice(hh * HB * C, (hh + 1) * HB * C)
                nc.vector.tensor_add(
                    out_sb[:, sl].rearrange("p (t c) -> p t c", c=C),
                    hd[:, sl].rearrange("p (t c) -> p t c", c=C),
                    bhb[:].unsqueeze(1).broadcast_to([128, HB, C]))
                nc.scalar.dma_start(out[:, sl], out_sb[:, sl])
    nc.compile()
    return nc


def _prep(x, edge_index, W1, b1, W2, b2, Wh, bh):
    x = np.asarray(x, np.float32)
    ei = np.asarray(edge_index, np.int64)
    src = np.concatenate([ei[0], np.arange(NPAD, dtype=np.int64)])
    dst = np.concatenate([ei[1], np.arange(NPAD, dtype=np.int64)])
    deg = np.bincount(dst, minlength=NPAD).astype(np.float32)
    dinv = 1.0 / np.sqrt(deg)

    xp = np.zeros((NPAD, D), np.float32)
    xp[:N] = x
    g0 = dinv[:, None] * xp
    g0_nm = g0.reshape(NSB, 128, D).transpose(1, 0, 2).reshape(128, NPAD)

    shared = {
        "g0": g0_nm.astype(FP8),
        "W1b": np.asarray(W1, np.float32).astype(BF16),
        "W2b": np.asarray(W2, np.float32).astype(BF16),
        "Wh": np.asarray(Wh, np.float32),
        "b1": np.asarray(b1, np.float32).reshape(D, 1),
        "b2": np.asarray(b2, np.float32).reshape(D, 1),
        "bhb": np.broadcast_to(np.asarray(bh, np.float32).reshape(1, C),
                               (128, C)).copy(),
        "eye": np.eye(128, dtype=np.float32).astype(BF16),
    }
    core = dst // DST
    sl, sbk = src % 128, src // 128
    in_maps = []
    for c in range(NCORES):
        m = core == c
        dloc = dst[m] - c * DST
        im = dict(shared, dinvb=np.broadcast_to(
            dinv[c * DST:(c + 1) * DST].reshape(1, DST), (128, DST)).copy())
        for ci, (off, ln) in enumerate(CH):
            m2 = (dloc >= off) & (dloc < off + ln)
            Ac = np.zeros((128, NSB * ln), np.float32)
            np.add.at(Ac, (sl[m][m2], sbk[m][m2] * ln + dloc[m2] - off), 1.0)
            im[f"A{ci}"] = Ac.astype(FP8)
        in_maps.append(im)
    return in_maps


def _run(inputs, trace=False):
    if "nc" not in _cache:
        _cache["nc"] = _build()
    in_maps = _prep(**inputs)
    res = run_bass_kernel_spmd(_cache["nc"], in_maps,
                               core_ids=list(range(NCORES)), trace=trace)
    # out is stored partition-major [128, NHK*C]; unpack to [DST, C]
    outs = []
    for c in range(NCORES):
        o = res.results[c]["out"].reshape(128, NHK, C)
        outs.append(o.transpose(1, 0, 2).reshape(DST, C))
    out = np.concatenate(outs, axis=0)[:N]
    return np.ascontiguousarray(out, dtype=np.float32), res


def kernel(**inputs):
    out, _ = _run(inputs, trace=False)
    return out


# revision 19
# speedup vs baseline: 1.1578x; 1.0260x over previous
"""GCN (2x GCNConv + linear head) on 8 NeuronCores via Bass/Tile.

v4 strategy (graph/data parallel, fp8 DoubleRow both layers):
  - Nodes padded to 10240 = 80 blocks of 128; core c owns dst range
    [c*1280, (c+1)*1280).
  - S = D^-1/2 (A+I) D^-1/2: A holds exact edge counts as dense fp8
    blocks; per-node scalings host-precomputed (g0 = dinv*x in fp8,
    dinvb broadcast tile).
  - Both aggregations run DoubleRow (paired 128-src-blocks, both
    operands fp8) at 2x PE rate. Features quantized to fp8 cost
    rel_err ~1.8e-2 (simulated) vs the 2e-2 gate; L2FP8=False falls
    back to bf16 layer 2 (~8e-3).
  - dinv_dst folded into the PSUM->SBUF cast (commutes with the W
    contraction), relu reads PSUM directly: short agg->cc_in chain.
  - g1 transposed to node-major BEFORE the collective; 3 pipelined
    AllGathers (one per dst chunk) with ~20us trigger->mesh latency
    hidden under layer-2 compute of earlier chunks.
  - A streamed in consumption order, split across both HWDGE rings.
  - Head accumulates all 10 blocks into one PSUM bank; output stored
    partition-major ([128, 10*C]) so the final DMA is contiguous and
    the host unpacks.
"""
import numpy as np
import ml_dtypes

import concourse.bass as bass
import concourse.mybir as mybir
import concourse.tile as tile
import concourse.bacc as bacc
from concourse.bass_utils import run_bass_kernel_spmd

FP8 = np.dtype(ml_dtypes.float8_e4m3)
BF16 = np.dtype(ml_dtypes.bfloat16)

N, E, D, C = 10000, 640000, 128, 40
NCORES = 8
NSB = 80                      # src blocks of 128
NPAD = NSB * 128              # 10240
DST = NPAD // NCORES          # 1280 dst nodes per core
CH = [(0, 512), (512, 512), (1024, 256)]
NHK = DST // 128              # head chunks of 128
L2FP8 = True                  # fp8 DoubleRow layer-2 aggregation: faster but
                              # rel_err 1.8e-2 vs 8e-3 (gate 2e-2, same seed)
AGS = [(0, 1024), (1024, 256)]  # allgather segments (cols of node-major g1)

_cache = {}


def _build():
    nc = bacc.Bacc("TRN2", target_bir_lowering=False, debug=False,
                   num_devices=NCORES)
    f32 = mybir.dt.float32
    bf16 = mybir.dt.bfloat16
    fp8 = mybir.dt.float8e4
    RELU = mybir.ActivationFunctionType.Relu
    DR = mybir.MatmulPerfMode.DoubleRow
    g1dt, g1np = (fp8, FP8) if L2FP8 else (bf16, BF16)

    g0_d = nc.dram_tensor("g0", [128, NPAD], fp8, kind="ExternalInput")
    W1b = nc.dram_tensor("W1b", [D, D], bf16, kind="ExternalInput")
    W2b = nc.dram_tensor("W2b", [D, D], bf16, kind="ExternalInput")
    Wh = nc.dram_tensor("Wh", [D, C], f32, kind="ExternalInput")
    b1 = nc.dram_tensor("b1", [D, 1], f32, kind="ExternalInput")
    b2 = nc.dram_tensor("b2", [D, 1], f32, kind="ExternalInput")
    bhb_d = nc.dram_tensor("bhb", [128, C], f32, kind="ExternalInput")
    eye_d = nc.dram_tensor("eye", [128, 128], bf16, kind="ExternalInput")
    dinvb_d = nc.dram_tensor("dinvb", [128, DST], f32, kind="ExternalInput")
    A_d = [nc.dram_tensor(f"A{ci}", [128, NSB * ln], fp8, kind="ExternalInput")
           for ci, (off, ln) in enumerate(CH)]
    out = nc.dram_tensor("out", [128, NHK * C], f32, kind="ExternalOutput")

    with tile.TileContext(nc) as tc:
        with (
            tc.tile_pool(name="big", bufs=1) as big,
            tc.tile_pool(name="sb", bufs=1) as sb,
            tc.tile_pool(name="tmpp", bufs=3) as tmpp,
            tc.tile_pool(name="psagg", bufs=3, space="PSUM") as psagg,
            tc.tile_pool(name="psz", bufs=1, space="PSUM") as psz,
            tc.tile_pool(name="pstr", bufs=2, space="PSUM") as pstr,
            tc.tile_pool(name="pshd", bufs=1, space="PSUM") as pshd,
            tc.tile_pool(name="dram", bufs=1, space="DRAM") as dram,
        ):
            # ---- warm-up collective: the first mesh starts at a fixed
            # ~77us firmware floor AND absorbs the ~14us cross-core
            # alignment; paying that inside a 128-byte mesh (~8us) beats
            # paying it inside the first real gather (measured +17us) ----
            warm_in = dram.tile([1, 128], bf16)
            warm_out = dram.tile([NCORES, 1, 128], bf16, addr_space="Shared")
            nc.gpsimd.collective_compute(
                "AllGather", mybir.AluOpType.bypass,
                replica_groups=[list(range(NCORES))],
                ins=[warm_in[:]], outs=[warm_out[:]])

            # ---- all input streams issued up front, consumption order,
            # split across both HWDGE rings (compute instructions later in
            # program order would head-of-line-block the ring queues) ----
            A_t = [big.tile([128, NSB * ln], fp8, name=f"At{ci}")
                   for ci, (off, ln) in enumerate(CH)]
            g0 = big.tile([128, NPAD], fp8)
            for p in range(4):  # even pieces -> sync, odd -> scalar
                s0, s1 = p * (NPAD // 4), (p + 1) * (NPAD // 4)
                (nc.sync if p % 2 == 0 else nc.scalar).dma_start(
                    g0[:, s0:s1], g0_d[:, s0:s1])
            dinvb = sb.tile([128, DST], f32)
            nc.scalar.dma_start(dinvb[:], dinvb_d[:, :])
            W1_t = sb.tile([D, D], bf16)
            nc.scalar.dma_start(W1_t[:], W1b[:, :])
            b1_t = sb.tile([D, 1], f32)
            nc.scalar.dma_start(b1_t[:], b1[:, :])
            eye_t = sb.tile([128, 128], bf16)
            nc.scalar.dma_start(eye_t[:], eye_d[:, :])

            def stream_A(ci, npc):
                ln = CH[ci][1]
                for q in range(npc):
                    s0 = q * (NSB // npc) * ln
                    s1 = (q + 1) * (NSB // npc) * ln
                    (nc.sync if q % 2 == 0 else nc.scalar).dma_start(
                        A_t[ci][:, s0:s1], A_d[ci][:, s0:s1])

            stream_A(0, 8)
            stream_A(1, 4)
            stream_A(2, 4)
            W2_t = sb.tile([D, D], bf16)
            nc.scalar.dma_start(W2_t[:], W2b[:, :])
            b2_t = sb.tile([D, 1], f32)
            nc.scalar.dma_start(b2_t[:], b2[:, :])
            Wh_t = sb.tile([D, C], f32)
            nc.scalar.dma_start(Wh_t[:], Wh[:, :])
            bhb = sb.tile([128, C], f32)
            nc.scalar.dma_start(bhb[:], bhb_d[:, :])

            # ---- layer 1 per chunk -> transpose -> allgather ----
            # collectives carry bf16 (fp8 collectives measured broken/slow)
            cc_src = sb.tile([128, DST], bf16)     # node-major g1
            cc_io = []

            def l1_chunk(ci):
                off, ln = CH[ci]
                agg = psagg.tile([128, 512], f32, tag="agg", name=f"agg1_{ci}")
                for pb in range(NSB // 2):
                    nc.tensor.matmul(
                        agg[:, :ln],
                        lhsT=g0[:, pb * 256:(pb + 1) * 256].rearrange(
                            "p (two f) -> p two f", two=2),
                        rhs=A_t[ci][:, pb * 2 * ln:(pb * 2 + 2) * ln].rearrange(
                            "p (two n) -> p two n", two=2),
                        start=(pb == 0), stop=(pb == NSB // 2 - 1),
                        perf_mode=DR)
                # dinv_dst folded here (commutes with the W1 contraction)
                acc = tmpp.tile([128, 512], bf16, tag="acc")
                nc.vector.tensor_mul(acc[:, :ln], agg[:, :ln],
                                     dinvb[:, off:off + ln])
                zps = psz.tile([128, 512], f32, tag="z")
                nc.tensor.matmul(zps[:, :ln], lhsT=W1_t[:], rhs=acc[:, :ln],
                                 start=True, stop=True)
                t2 = tmpp.tile([128, 512], f32, tag="t2")
                nc.scalar.activation(t2[:, :ln], zps[:, :ln], RELU,
                                     bias=b1_t[:, 0:1], scale=1.0)
                g1c = tmpp.tile([128, 512], bf16, tag="g1c")
                nc.vector.tensor_mul(g1c[:, :ln], t2[:, :ln],
                                     dinvb[:, off:off + ln])
                nt = ln // 128
                # transpose in bf16 (fp8 PE-transpose needs stride-2 out);
                # the PSUM->SBUF copy below converts to g1dt
                trp = pstr.tile([128, 4, 128], bf16, tag="tr")
                for t in range(nt):
                    nc.tensor.transpose(trp[:, t, :],
                                        g1c[:, t * 128:(t + 1) * 128], eye_t[:])
                nc.vector.tensor_copy(
                    cc_src[:, off:off + ln].rearrange(
                        "p (t f) -> p t f", f=128),
                    trp[:, :nt, :])

            def fire_ag(qi):
                o0, w = AGS[qi]
                cc_in = dram.tile([128, w], bf16, name=f"cc_in{qi}")
                cc_out = dram.tile([NCORES, 128, w], bf16,
                                   addr_space="Shared", name=f"cc_out{qi}")
                nc.sync.dma_start(cc_in[:], cc_src[:, o0:o0 + w])
                nc.gpsimd.collective_compute(
                    "AllGather", mybir.AluOpType.bypass,
                    replica_groups=[list(range(NCORES))],
                    ins=[cc_in[:]], outs=[cc_out[:]])
                cc_io.append((cc_in, cc_out))

            l1_chunk(0)
            l1_chunk(1)
            fire_ag(0)
            l1_chunk(2)
            fire_ag(1)

            # ---- layer 2: gathered node-major slabs feed matmuls ----
            agg2 = [psagg.tile([128, 512], f32, tag="agg", name=f"agg2_{c2}")
                    for c2 in range(len(CH))]
            slabs = []
            for qi, (o0, w) in enumerate(AGS):
                slab = sb.tile([128, NCORES * w], bf16, name=f"slab{qi}")
                slab8 = sb.tile([128, NCORES * w], fp8,
                                name=f"slab8_{qi}") if L2FP8 else None
                # fine pieces across both rings + per-piece fp8 casts so
                # layer-2 matmuls unblock as data trickles in
                npc = 4 if qi == 0 else 2
                PC = NCORES // npc
                for q in range(npc):
                    sl = slice(q * PC * w, (q + 1) * PC * w)
                    (nc.sync if q % 2 == 0 else nc.scalar).dma_start(
                        slab[:, sl].rearrange("p (r d) -> p r d", d=w),
                        cc_io[qi][1][q * PC:(q + 1) * PC, :, :].rearrange(
                            "r p d -> p r d"))
                    if L2FP8:
                        nc.vector.tensor_copy(slab8[:, sl], slab[:, sl])
                slabs.append(slab8 if L2FP8 else slab)

            # global src block for (qi, r, t): r*NHK + o0/128 + t
            k = 0
            nblk = [w // 128 for _, w in AGS]
            if L2FP8:
                npairs = NSB // 2
                for qi, (o0, w) in enumerate(AGS):
                    for r in range(NCORES):
                        for u in range(nblk[qi] // 2):
                            lhsT = slabs[qi][:, r * w + u * 256:
                                             r * w + (u + 1) * 256].rearrange(
                                "p (two f) -> p two f", two=2)
                            sb_g = r * NHK + o0 // 128 + 2 * u
                            for c2, (off2, ln2) in enumerate(CH):
                                nc.tensor.matmul(
                                    agg2[c2][:, :ln2], lhsT=lhsT,
                                    rhs=A_t[c2][:, sb_g * ln2:
                                                (sb_g + 2) * ln2].rearrange(
                                        "p (two n) -> p two n", two=2),
                                    start=(k == 0), stop=(k == npairs - 1),
                                    perf_mode=DR)
                            k += 1
            else:
                for qi, (o0, w) in enumerate(AGS):
                    for r in range(NCORES):
                        for t in range(nblk[qi]):
                            lhsT = slabs[qi][:, r * w + t * 128:
                                             r * w + (t + 1) * 128]
                            sb_g = r * NHK + o0 // 128 + t
                            for c2, (off2, ln2) in enumerate(CH):
                                nc.tensor.matmul(
                                    agg2[c2][:, :ln2], lhsT=lhsT,
                                    rhs=A_t[c2][:, sb_g * ln2:(sb_g + 1) * ln2],
                                    start=(k == 0), stop=(k == NSB - 1))
                            k += 1

            h2 = sb.tile([128, DST], f32)
            for c2, (off2, ln2) in enumerate(CH):
                acc = tmpp.tile([128, 512], bf16, tag="acc")
                nc.vector.tensor_mul(acc[:, :ln2], agg2[c2][:, :ln2],
                                     dinvb[:, off2:off2 + ln2])
                zps = psz.tile([128, 512], f32, tag="z")
                nc.tensor.matmul(zps[:, :ln2], lhsT=W2_t[:], rhs=acc[:, :ln2],
                                 start=True, stop=True)
                nc.scalar.activation(h2[:, off2:off2 + ln2], zps[:, :ln2],
                                     RELU, bias=b2_t[:, 0:1], scale=1.0)

            # ---- head: all 10 blocks into one PSUM bank; bias-add and
            # output DMA split in halves to pipeline the tail ----
            hd = pshd.tile([128, NHK * C], f32)
            out_sb = sb.tile([128, NHK * C], f32)
            HB = NHK // 2
            for hh in range(2):
                for hk in range(hh * HB, (hh + 1) * HB):
                    nc.tensor.matmul(hd[:, hk * C:(hk + 1) * C],
                                     lhsT=h2[:, hk * 128:(hk + 1) * 128],
                                     rhs=Wh_t[:], start=True, stop=True)
                sl = slice(hh * HB * C, (hh + 1) * HB * C)
                nc.vector.tensor_add(
                    out_sb[:, sl].rearrange("p (t c) -> p t c", c=C),
                    hd[:, sl].rearrange("p (t c) -> p t c", c=C),
                    bhb[:].unsqueeze(1).broadcast_to([128, HB, C]))
                nc.scalar.dma_start(out[:, sl], out_sb[:, sl])
    nc.compile()
    return nc


def _prep(x, edge_index, W1, b1, W2, b2, Wh, bh):
    x = np.asarray(x, np.float32)
    ei = np.asarray(edge_index, np.int64)
    src = np.concatenate([ei[0], np.arange(NPAD, dtype=np.int64)])
    dst = np.concatenate([ei[1], np.arange(NPAD, dtype=np.int64)])
    deg = np.bincount(dst, minlength=NPAD).astype(np.float32)
    dinv = 1.0 / np.sqrt(deg)

    xp = np.zeros((NPAD, D), np.float32)
    xp[:N] = x
    g0 = dinv[:, None] * xp
    g0_nm = g0.reshape(NSB, 128, D).transpose(1, 0, 2).reshape(128, NPAD)

    shared = {
        "g0": g0_nm.astype(FP8),
        "W1b": np.asarray(W1, np.float32).astype(BF16),
        "W2b": np.asarray(W2, np.float32).astype(BF16),
        "Wh": np.asarray(Wh, np.float32),
        "b1": np.asarray(b1, np.float32).reshape(D, 1),
        "b2": np.asarray(b2, np.float32).reshape(D, 1),
        "bhb": np.broadcast_to(np.asarray(bh, np.float32).reshape(1, C),
                               (128, C)).copy(),
        "eye": np.eye(128, dtype=np.float32).astype(BF16),
    }
    core = dst // DST
    sl, sbk = src % 128, src // 128
    in_maps = []
    for c in range(NCORES):
        m = core == c
        dloc = dst[m] - c * DST
        im = dict(shared, dinvb=np.broadcast_to(
            dinv[c * DST:(c + 1) * DST].reshape(1, DST), (128, DST)).copy())
        for ci, (off, ln) in enumerate(CH):
            m2 = (dloc >= off) & (dloc < off + ln)
            Ac = np.zeros((128, NSB * ln), np.float32)
            np.add.at(Ac, (sl[m][m2], sbk[m][m2] * ln + dloc[m2] - off), 1.0)
            im[f"A{ci}"] = Ac.astype(FP8)
        in_maps.append(im)
    return in_maps


def _run(inputs, trace=False):
    if "nc" not in _cache:
        _cache["nc"] = _build()
    in_maps = _prep(**inputs)
    res = run_bass_kernel_spmd(_cache["nc"], in_maps,
                               core_ids=list(range(NCORES)), trace=trace)
    # out is stored partition-major [128, NHK*C]; unpack to [DST, C]
    outs = []
    for c in range(NCORES):
        o = res.results[c]["out"].reshape(128, NHK, C)
        outs.append(o.transpose(1, 0, 2).reshape(DST, C))
    out = np.concatenate(outs, axis=0)[:N]
    return np.ascontiguousarray(out, dtype=np.float32), res


def kernel(**inputs):
    out, _ = _run(inputs, trace=False)
    return out
